# revision 11
# baseline (speedup 1.0000x reference)
"""Distributed Trainium2 (Bass) kernel for nn_AtomEmbedder (2-layer GCN + embed).

Strategy (8 NeuronCores, node-sharded):
  - Nodes padded to 50176 = 8 * 6272; core c owns dst rows [c*6272, (c+1)*6272).
  - h0 = relu(x @ We + be) computed feature-major per shard (no transposes).
  - Per GCN layer l:
      g = dis * (h @ Wl)  (node-major shard, dense matmuls on PE)
      AllGather g -> full 50176-row table in DRAM
      edge scatter: for each 128-dst-node window, accumulate in PSUM
        sum_e g[src_e] via dma_gather (token stream sorted by window) +
        one-hot matmul (S^T @ gathered), plus self-loop (identity matmul on
        own shard rows) and bias folded as a K=1 matmul of sqrt(deg) x b.
      epilogue: out = act(dis * psum) in one ScalarE op per window.
  - Layer-1 windows are transposed on PE into feature-major h1T so layer-2
    dense matmuls start immediately (hidden under layer-1 gathers).

The token gather is Q7-descriptor-rate-bound (~8.5 ns/token); everything
else (PE matmuls, DVE one-hot builds, ACT epilogues, dense DMA, AllGather)
is scheduled to hide underneath it.
"""

import numpy as np
import ml_dtypes

BF16 = ml_dtypes.bfloat16
N_NODES = 50000
N_EDGES = 300000
F_IN = 11
D = 256
NCORES = 8
NSH = 6272            # nodes per core (50176 total, padded)
NTAB = NCORES * NSH   # 50176
HALF = NTAB // 2
AWIN = 25             # windows per core in table half A
AROWS = AWIN * 128    # 3200 rows/core  -> A table 25600 rows (< 32768, int16-safe)
BROWS = NSH - AROWS   # 3072 rows/core  -> B table 24576 rows
NWIN = NSH // 128     # 49 windows per core
CALL = 2048           # tokens per dma_gather call
P = 128

_CACHE = {}


def _edge_plan(src, dst):
    """Build the SPMD-uniform token-stream / segment structure plus per-core
    index data. Returns (meta, per_core_arrays)."""
    src = src.astype(np.int64)
    dst = dst.astype(np.int64)
    core = dst // NSH
    dloc = dst % NSH
    win = dloc // 128
    rsrc = src % NSH
    csrc = src // NSH
    half = (rsrc >= AROWS).astype(np.int64)
    tokval = np.where(half == 0, csrc * AROWS + rsrc,
                      csrc * BROWS + (rsrc - AROWS))

    # counts[c, w, h]
    counts = np.zeros((NCORES, NWIN, 2), dtype=np.int64)
    np.add.at(counts, (core, win, half), 1)
    gcnt = counts.max(axis=0)              # [NWIN, 2] uniform per-window counts

    # stream layout per half: windows in order, gcnt tokens each
    streams = {}
    for h in (0, 1):
        lens = gcnt[:, h]
        total = int(lens.sum())
        ncalls = max(1, -(-total // CALL))
        padded = ncalls * CALL
        last = total - (ncalls - 1) * CALL
        call_sizes = [CALL] * (ncalls - 1) + [max(128, -(-last // 128) * 128)]
        win_start = np.zeros(NWIN + 1, dtype=np.int64)
        win_start[1:] = np.cumsum(lens)
        streams[h] = dict(lens=lens, total=total, ncalls=ncalls, padded=padded,
                          win_start=win_start, call_sizes=call_sizes)

    # segments: (half, chunk, window) for every chunk x window overlap
    segs = []          # list of (half, chunk_idx, window)
    win_segs = {0: [[] for _ in range(NWIN)], 1: [[] for _ in range(NWIN)]}
    for h in (0, 1):
        ws = streams[h]["win_start"]
        nchunk = streams[h]["padded"] // 128
        for w in range(NWIN):
            a, b = int(ws[w]), int(ws[w + 1])
            if a == b:
                continue
            for ch in range(a // 128, (b - 1) // 128 + 1):
                win_segs[h][w].append(len(segs))
                segs.append((h, ch, w))
        streams[h]["nchunk"] = nchunk
    nseg = len(segs)

    # per-core data: token indices (int16, per half) and dstloc columns
    per_core = []
    for c in range(NCORES):
        m = core == c
        s_c, w_c, h_c, dl_c = tokval[m], win[m], half[m], dloc[m]
        core_tok = {}
        for h in (0, 1):
            st = streams[h]
            tok = np.zeros(st["padded"], dtype=np.int16)   # pad -> row 0
            dstl = np.full(st["padded"], -999.0, dtype=np.float32)
            mh = h_c == h
            s_h, w_h, dl_h = s_c[mh], w_c[mh], dl_c[mh]
            order = np.argsort(w_h, kind="stable")
            s_h, w_h, dl_h = s_h[order], w_h[order], dl_h[order]
            # place each window's tokens at its global window start
            cnts = np.bincount(w_h, minlength=NWIN)
            pos = st["win_start"][w_h] + (np.arange(len(w_h))
                                          - np.repeat(np.cumsum(cnts) - cnts, cnts))
            tok[pos] = s_h.astype(np.int16)
            dstl[pos] = dl_h.astype(np.float32)
            core_tok[h] = (tok, dstl)
        # wrap idx tensors: [128, ncols]; within call k (2048 tokens ->
        # 128 cols), token j -> idxs[j % 16, k*128 + j // 16]
        idx_w = {}
        for h in (0, 1):
            tok = core_tok[h][0]
            st = streams[h]
            cols = []
            for k in range(st["ncalls"]):
                blk = tok[k * CALL:(k + 1) * CALL].reshape(128, 16).T  # [16,128]
                cols.append(blk)
            w16 = np.concatenate(cols, axis=1)          # [16, ncalls*128]
            idx_w[h] = np.tile(w16, (8, 1)).copy()      # [128, ncalls*128]
        # dstloc tensor [128, nseg]: segment s=(h, ch, w) -> column of
        # dloc - w*128 for the chunk's 128 tokens
        dstloc = np.full((128, nseg), -999.0, dtype=np.float32)
        for si, (h, ch, w) in enumerate(segs):
            dstl = core_tok[h][1]
            col = dstl[ch * 128:(ch + 1) * 128] - w * 128
            col[col < -500] = -999.0
            dstloc[:, si] = col
        per_core.append(dict(idx_lo=idx_w[0], idx_hi=idx_w[1], dstloc=dstloc))

    meta = dict(streams=streams, segs=segs, win_segs=win_segs, nseg=nseg)
    return meta, per_core


def _build_program(meta):
    import concourse.bass as bass
    import concourse.bacc as bacc
    import concourse.tile as tile
    import concourse.mybir as mybir

    f32 = mybir.dt.float32
    bf = mybir.dt.bfloat16
    i16 = mybir.dt.int16
    AF = mybir.ActivationFunctionType

    st_lo, st_hi = meta["streams"][0], meta["streams"][1]
    nseg = meta["nseg"]
    segs = meta["segs"]
    win_segs = meta["win_segs"]
    ncalls = {0: st_lo["ncalls"], 1: st_hi["ncalls"]}
    idx_cols = {h: ncalls[h] * 128 for h in (0, 1)}

    nc = bacc.Bacc("TRN2", target_bir_lowering=False, debug=False,
                   num_devices=NCORES)

    # ---- external I/O (per-core shards) ----
    xT = nc.dram_tensor("xT", [F_IN, NSH], bf, kind="ExternalInput")
    We = nc.dram_tensor("We", [F_IN, D], bf, kind="ExternalInput")
    beW = nc.dram_tensor("beW", [128, 2], f32, kind="ExternalInput")
    W1 = nc.dram_tensor("W1", [D, D], bf, kind="ExternalInput")
    W2 = nc.dram_tensor("W2", [D, D], bf, kind="ExternalInput")
    b1 = nc.dram_tensor("b1", [1, D], bf, kind="ExternalInput")
    b2 = nc.dram_tensor("b2", [1, D], bf, kind="ExternalInput")
    disw = nc.dram_tensor("disw", [128, NWIN], f32, kind="ExternalInput")
    sqd = nc.dram_tensor("sqd", [1, NSH], bf, kind="ExternalInput")
    iota = nc.dram_tensor("iota", [128, 128], f32, kind="ExternalInput")
    ident = nc.dram_tensor("ident", [128, 128], f32, kind="ExternalInput")
    identb = nc.dram_tensor("identb", [128, 128], bf, kind="ExternalInput")
    idx_lo = nc.dram_tensor("idx_lo", [128, idx_cols[0]], i16, kind="ExternalInput")
    idx_hi = nc.dram_tensor("idx_hi", [128, idx_cols[1]], i16, kind="ExternalInput")
    dstloc = nc.dram_tensor("dstloc", [128, nseg], f32, kind="ExternalInput")
    out = nc.dram_tensor("out", [NSH, D], f32, kind="ExternalOutput")

    with tile.TileContext(nc) as tc:
        with (
            tc.tile_pool(name="const", bufs=1) as constp,
            tc.tile_pool(name="hT", bufs=1) as hTp,
            tc.tile_pool(name="dram", bufs=1, space="DRAM") as dramp,
            tc.tile_pool(name="glo", bufs=3) as glop,
            tc.tile_pool(name="ghi", bufs=3) as ghip,
            tc.tile_pool(name="acc", bufs=4, space="PSUM") as accp,
            tc.tile_pool(name="tps", bufs=2, space="PSUM") as tpsp,
            tc.tile_pool(name="sg", bufs=3) as sgp,
            tc.tile_pool(name="ob", bufs=3) as obp,
            tc.tile_pool(name="sm", bufs=4) as smp,
        ):
            # ---- load constants ----
            xT_sb = constp.tile([F_IN, NSH], bf)
            We_sb = constp.tile([F_IN, D], bf)
            beW_sb = constp.tile([128, 2], f32)
            W1_sb = constp.tile([128, 2, D], bf)
            W2_sb = constp.tile([128, 2, D], bf)
            b1_sb = constp.tile([1, D], bf)
            b2_sb = constp.tile([1, D], bf)
            disw_sb = constp.tile([128, NWIN], f32)
            sqd_sb = constp.tile([1, NSH], bf)
            iota_sb = constp.tile([128, 128], f32)
            ident_sb = constp.tile([128, 128], f32)
            identb_sb = constp.tile([128, 128], bf)
            ilo_sb = constp.tile([128, idx_cols[0]], i16)
            ihi_sb = constp.tile([128, idx_cols[1]], i16)
            dstloc_sb = constp.tile([128, nseg], f32)

            nc.sync.dma_start(xT_sb[:], xT[:])
            nc.sync.dma_start(We_sb[:], We[:])
            nc.sync.dma_start(beW_sb[:], beW[:])
            nc.sync.dma_start(W1_sb[:, 0, :], W1[0:128, :])
            nc.sync.dma_start(W1_sb[:, 1, :], W1[128:256, :])
            nc.sync.dma_start(W2_sb[:, 0, :], W2[0:128, :])
            nc.sync.dma_start(W2_sb[:, 1, :], W2[128:256, :])
            nc.sync.dma_start(b1_sb[:], b1[:])
            nc.sync.dma_start(b2_sb[:], b2[:])
            nc.sync.dma_start(disw_sb[:], disw[:])
            nc.sync.dma_start(sqd_sb[:], sqd[:])
            nc.sync.dma_start(iota_sb[:], iota[:])
            nc.sync.dma_start(ident_sb[:], ident[:])
            nc.sync.dma_start(identb_sb[:], identb[:])
            nc.sync.dma_start(ilo_sb[:], idx_lo[:])
            nc.sync.dma_start(ihi_sb[:], idx_hi[:])
            nc.sync.dma_start(dstloc_sb[:], dstloc[:])

            h0T = hTp.tile([128, 2, NSH], bf)
            h1T = hTp.tile([128, 2, NSH], bf)

            # ---- embed: h0T = relu(We^T x^T + be), feature-major ----
            slabs = [(s, min(s + 512, NSH)) for s in range(0, NSH, 512)]
            for k in (0, 1):
                for (a, b) in slabs:
                    ps = accp.tile([128, 512], f32, tag="acc")
                    nc.tensor.matmul(ps[:, :b - a], lhsT=We_sb[:, k * 128:(k + 1) * 128],
                                     rhs=xT_sb[:, a:b], start=True, stop=True)
                    nc.scalar.activation(h0T[:, k, a:b], ps[:, :b - a], AF.Relu,
                                         bias=beW_sb[:, k:k + 1], scale=1.0)

            cc_in = {}
            cc_out = {}
            for l in (1, 2):
                cc_in[l] = {0: dramp.tile([AROWS, D], bf, name=f"ccinA{l}"),
                            1: dramp.tile([BROWS, D], bf, name=f"ccinB{l}")}
                cc_out[l] = {0: dramp.tile([NCORES * AROWS, D], bf,
                                           name=f"ccoutA{l}", addr_space="Shared"),
                             1: dramp.tile([NCORES * BROWS, D], bf,
                                           name=f"ccoutB{l}", addr_space="Shared")}
            accA = hTp.tile([128, NWIN, D], bf, name="accA")

            def dense(l, w, hT, W_sb):
                """g tile for window w of layer l -> SBUF + DMA to cc_in[l]."""
                ps = accp.tile([128, D], f32, tag="acc", name=f"dps{l}_{w}")
                for k in (0, 1):
                    nc.tensor.matmul(ps[:], lhsT=hT[:, k, w * 128:(w + 1) * 128],
                                     rhs=W_sb[:, k, :], start=(k == 0), stop=(k == 1))
                gt = obp.tile([128, D], bf, tag="ob", name=f"g{l}_{w}")
                nc.scalar.activation(gt[:], ps[:], AF.Copy, bias=0.0,
                                     scale=disw_sb[:, w:w + 1])
                if w < AWIN:
                    nc.sync.dma_start(cc_in[l][0][w * 128:(w + 1) * 128, :], gt[:])
                else:
                    ww = w - AWIN
                    nc.sync.dma_start(cc_in[l][1][ww * 128:(ww + 1) * 128, :], gt[:])

            def allgather(l, h):
                nc.gpsimd.collective_compute(
                    "AllGather", mybir.AluOpType.bypass,
                    replica_groups=[list(range(NCORES))],
                    ins=[cc_in[l][h][:]], outs=[cc_out[l][h][:]])

            for w in range(NWIN):
                dense(1, w, h0T, W1_sb)
                if w == AWIN - 1:
                    allgather(1, 0)
            allgather(1, 1)

            def edge_phase(l, b_sb, post_b):
                """Two-pass edge scatter for layer l.

                Pass A: psum = self + bias + A-half segments -> accA (bf16).
                Pass B: psum = B-half segments + I @ accA -> post_b(w, psum).
                Gather calls for half B sit after all half-A calls in the
                gpsimd queue, so AllGather of table half B hides under the
                half-A gathers."""
                gt_tiles = {0: {}, 1: {}}
                emitted = {0: 0, 1: 0}
                idx_sb = {0: ilo_sb, 1: ihi_sb}
                pool = {0: glop, 1: ghip}
                sizes = {0: st_lo["call_sizes"], 1: st_hi["call_sizes"]}

                def emit_call(h):
                    k = emitted[h]
                    nidx = sizes[h][k]
                    g = pool[h].tile([128, nidx // 128, D], bf, tag=f"g{h}",
                                     name=f"L{l}{'ab'[h]}{k}")
                    nc.gpsimd.dma_gather(
                        out_ap=g[:], in_ap=cc_out[l][h][:],
                        idxs_ap=idx_sb[h][:, k * 128:k * 128 + nidx // 16],
                        num_idxs=nidx, num_idxs_reg=nidx, elem_size=D,
                        single_packet=False)
                    gt_tiles[h][k] = g
                    emitted[h] += 1

                def seg_mms(h, w, ps, first_start, last_stop=False):
                    lst = win_segs[h][w]
                    for j, si in enumerate(lst):
                        _, ch, _ = segs[si]
                        call_k, cj = ch * 128 // CALL, (ch * 128 % CALL) // 128
                        S = smp.tile([128, 128], bf, tag="sm", name=f"S{l}_{si}")
                        nc.vector.tensor_tensor(
                            out=S[:],
                            in0=dstloc_sb[:, si:si + 1].to_broadcast([128, 128]),
                            in1=iota_sb[:],
                            op=mybir.AluOpType.is_equal)
                        nc.tensor.matmul(ps[:], lhsT=S[:],
                                         rhs=gt_tiles[h][call_k][:, cj, :],
                                         start=(first_start and j == 0),
                                         stop=(last_stop and j == len(lst) - 1))

                # ---- pass A ----
                for w in range(NWIN):
                    need = max((segs[si][1] * 128 // CALL + 1
                                for si in win_segs[0][w]), default=0)
                    while emitted[0] < need:
                        emit_call(0)
                    ps = accp.tile([128, D], f32, tag="acc", name=f"pa{l}_{w}")
                    sgt = sgp.tile([128, D], bf, tag="sg", name=f"sg{l}_{w}")
                    if w < AWIN:
                        nc.sync.dma_start(sgt[:], cc_in[l][0][w * 128:(w + 1) * 128, :])
                    else:
                        ww = w - AWIN
                        nc.sync.dma_start(sgt[:], cc_in[l][1][ww * 128:(ww + 1) * 128, :])
                    nc.tensor.matmul(ps[:], lhsT=identb_sb[:], rhs=sgt[:],
                                     start=True, stop=False)
                    nc.tensor.matmul(ps[:], lhsT=sqd_sb[0:1, w * 128:(w + 1) * 128],
                                     rhs=b_sb[0:1, :], start=False,
                                     stop=len(win_segs[0][w]) == 0)
                    seg_mms(0, w, ps, False, last_stop=True)
                    nc.vector.tensor_copy(accA[:, w, :], ps[:])
                # ---- pass B ----
                for w in range(NWIN):
                    need = max((segs[si][1] * 128 // CALL + 1
                                for si in win_segs[1][w]), default=0)
                    while emitted[1] < need:
                        emit_call(1)
                    ps = accp.tile([128, D], f32, tag="acc", name=f"pb{l}_{w}")
                    seg_mms(1, w, ps, True, last_stop=False)
                    nc.tensor.matmul(ps[:], lhsT=identb_sb[:], rhs=accA[:, w, :],
                                     start=len(win_segs[1][w]) == 0, stop=True)
                    post_b(w, ps)

            # ---- layer 1: edge phase -> h1T (transposed) + dense2 ----
            def l1_post(w, ps):
                ot = obp.tile([128, D], f32, tag="ob", name=f"h1_{w}")
                nc.scalar.activation(ot[:], ps[:], AF.Relu, bias=0.0,
                                     scale=disw_sb[:, w:w + 1])
                for k in (0, 1):
                    tp = tpsp.tile([128, 128], f32, tag="tp", name=f"tp{w}_{k}")
                    nc.tensor.transpose(tp[:], ot[:, k * 128:(k + 1) * 128],
                                        ident_sb[:])
                    nc.vector.tensor_copy(h1T[:, k, w * 128:(w + 1) * 128], tp[:])
                dense(2, w, h1T, W2_sb)
                if w == AWIN - 1:
                    allgather(2, 0)
                elif w == NWIN - 1:
                    allgather(2, 1)

            edge_phase(1, b1_sb, l1_post)

            # ---- layer 2 ----
            def l2_post(w, ps):
                ot = obp.tile([128, D], f32, tag="ob", name=f"o_{w}")
                nc.scalar.activation(ot[:], ps[:], AF.Copy, bias=0.0,
                                     scale=disw_sb[:, w:w + 1])
                nc.sync.dma_start(out[w * 128:(w + 1) * 128, :], ot[:])

            edge_phase(2, b2_sb, l2_post)

    nc.compile()
    return nc


def _prep_inputs(x, edge_index, W_embed, b_embed, W1, b1, W2, b2):
    src0, dst0 = np.asarray(edge_index[0]).astype(np.int64), \
        np.asarray(edge_index[1]).astype(np.int64)

    # degree-balanced node relabeling: deal degree-sorted nodes round-robin
    # into the 392 global windows so per-window edge counts (and the
    # max-over-cores stream padding) are nearly uniform.
    deg_orig = np.zeros(NTAB, dtype=np.int64)
    np.add.at(deg_orig, dst0, 1)
    order = np.argsort(-deg_orig, kind="stable")
    nwin_g = NTAB // 128
    ar = np.arange(NTAB)
    newid = np.empty(NTAB, dtype=np.int64)
    newid[order] = (ar % nwin_g) * 128 + ar // nwin_g
    src, dst = newid[src0], newid[dst0]
    meta, per_core = _edge_plan(src, dst)

    deg_d = 1.0 + np.zeros(NTAB, dtype=np.float64)
    np.add.at(deg_d, dst, 1)
    deg = deg_d
    dis = (1.0 / np.sqrt(deg)).astype(np.float32)
    sq = np.sqrt(deg).astype(np.float32)

    xpad = np.zeros((NTAB, F_IN), dtype=np.float32)
    xpad[newid[:N_NODES]] = x
    xT_full = np.ascontiguousarray(xpad.T)            # [11, NTAB]

    beW = np.asarray(b_embed, dtype=np.float32).reshape(2, 128).T.copy()  # [128,2]
    iota = np.tile(np.arange(128, dtype=np.float32), (128, 1))
    ident = np.eye(128, dtype=np.float32)

    in_maps = []
    for c in range(NCORES):
        sl = slice(c * NSH, (c + 1) * NSH)
        disw = dis[sl].reshape(NWIN, 128).T.copy()    # [128, NWIN]
        in_maps.append({
            "xT": np.ascontiguousarray(xT_full[:, sl]).astype(BF16),
            "We": np.asarray(W_embed, dtype=np.float32).astype(BF16),
            "beW": beW,
            "W1": np.asarray(W1, dtype=np.float32).astype(BF16),
            "W2": np.asarray(W2, dtype=np.float32).astype(BF16),
            "b1": np.asarray(b1, dtype=np.float32).reshape(1, D).astype(BF16),
            "b2": np.asarray(b2, dtype=np.float32).reshape(1, D).astype(BF16),
            "disw": disw,
            "sqd": sq[sl].reshape(1, NSH).astype(BF16),
            "iota": iota,
            "ident": ident,
            "identb": ident.astype(BF16),
            "idx_lo": per_core[c]["idx_lo"],
            "idx_hi": per_core[c]["idx_hi"],
            "dstloc": per_core[c]["dstloc"],
        })
    return meta, in_maps, newid


def kernel(x, edge_index, W_embed, b_embed, W1, b1, W2, b2, _trace=False):
    from concourse.bass_utils import run_bass_kernel_spmd

    meta, in_maps, newid = _prep_inputs(x, edge_index, W_embed, b_embed,
                                        W1, b1, W2, b2)
    key = (meta["streams"][0]["ncalls"], meta["streams"][1]["ncalls"],
           meta["nseg"], tuple(meta["streams"][0]["lens"].tolist()),
           tuple(meta["streams"][1]["lens"].tolist()))
    if key not in _CACHE:
        _CACHE.clear()
        _CACHE[key] = _build_program(meta)
    nc = _CACHE[key]

    res = run_bass_kernel_spmd(nc, in_maps, core_ids=list(range(NCORES)),
                               trace=_trace)
    full = np.concatenate([res.results[c]["out"] for c in range(NCORES)], axis=0)
    kernel._last_exec_ns = res.exec_time_ns
    return full[newid[:N_NODES]].astype(np.float32)


# revision 12
# speedup vs baseline: 1.0662x; 1.0662x over previous
"""Distributed Trainium2 (Bass) kernel for nn_AtomEmbedder (2-layer GCN + embed).

Strategy (8 NeuronCores, node-sharded):
  - Nodes padded to 50176 = 8 * 6272; core c owns dst rows [c*6272, (c+1)*6272).
  - h0 = relu(x @ We + be) computed feature-major per shard (no transposes).
  - Per GCN layer l:
      g = dis * (h @ Wl)  (node-major shard, dense matmuls on PE)
      AllGather g -> full 50176-row table in DRAM
      edge scatter: for each 128-dst-node window, accumulate in PSUM
        sum_e g[src_e] via dma_gather (token stream sorted by window) +
        one-hot matmul (S^T @ gathered), plus self-loop (identity matmul on
        own shard rows) and bias folded as a K=1 matmul of sqrt(deg) x b.
      epilogue: out = act(dis * psum) in one ScalarE op per window.
  - Layer-1 windows are transposed on PE into feature-major h1T so layer-2
    dense matmuls start immediately (hidden under layer-1 gathers).

The token gather is Q7-descriptor-rate-bound (~8.5 ns/token); everything
else (PE matmuls, DVE one-hot builds, ACT epilogues, dense DMA, AllGather)
is scheduled to hide underneath it.
"""

import numpy as np
import ml_dtypes

BF16 = ml_dtypes.bfloat16
N_NODES = 50000
N_EDGES = 300000
F_IN = 11
D = 256
NCORES = 8
NSH = 6272            # nodes per core (50176 total, padded)
NTAB = NCORES * NSH   # 50176
HALF = NTAB // 2
AWIN = 25             # windows per core in table half A
AROWS = AWIN * 128    # 3200 rows/core  -> A table 25600 rows (< 32768, int16-safe)
BROWS = NSH - AROWS   # 3072 rows/core  -> B table 24576 rows
NWIN = NSH // 128     # 49 windows per core
CALL = 2048           # tokens per dma_gather call
P = 128

_CACHE = {}


def _edge_plan(src, dst):
    """Build the SPMD-uniform token-stream / segment structure plus per-core
    index data. Returns (meta, per_core_arrays)."""
    src = src.astype(np.int64)
    dst = dst.astype(np.int64)
    core = dst // NSH
    dloc = dst % NSH
    win = dloc // 128
    rsrc = src % NSH
    csrc = src // NSH
    half = (rsrc >= AROWS).astype(np.int64)
    tokval = np.where(half == 0, csrc * AROWS + rsrc,
                      csrc * BROWS + (rsrc - AROWS))

    # counts[c, w, h]
    counts = np.zeros((NCORES, NWIN, 2), dtype=np.int64)
    np.add.at(counts, (core, win, half), 1)
    gcnt = counts.max(axis=0)              # [NWIN, 2] uniform per-window counts

    # stream layout per half: windows in order, gcnt tokens each
    streams = {}
    for h in (0, 1):
        lens = gcnt[:, h]
        total = int(lens.sum())
        ncalls = max(1, -(-total // CALL))
        padded = ncalls * CALL
        last = total - (ncalls - 1) * CALL
        call_sizes = [CALL] * (ncalls - 1) + [max(128, -(-last // 128) * 128)]
        win_start = np.zeros(NWIN + 1, dtype=np.int64)
        win_start[1:] = np.cumsum(lens)
        streams[h] = dict(lens=lens, total=total, ncalls=ncalls, padded=padded,
                          win_start=win_start, call_sizes=call_sizes)

    # segments: (half, chunk, window) for every chunk x window overlap
    segs = []          # list of (half, chunk_idx, window)
    win_segs = {0: [[] for _ in range(NWIN)], 1: [[] for _ in range(NWIN)]}
    for h in (0, 1):
        ws = streams[h]["win_start"]
        nchunk = streams[h]["padded"] // 128
        for w in range(NWIN):
            a, b = int(ws[w]), int(ws[w + 1])
            if a == b:
                continue
            for ch in range(a // 128, (b - 1) // 128 + 1):
                win_segs[h][w].append(len(segs))
                segs.append((h, ch, w))
        streams[h]["nchunk"] = nchunk
    nseg = len(segs)

    # per-core data: token indices (int16, per half) and dstloc columns
    per_core = []
    for c in range(NCORES):
        m = core == c
        s_c, w_c, h_c, dl_c = tokval[m], win[m], half[m], dloc[m]
        core_tok = {}
        for h in (0, 1):
            st = streams[h]
            tok = np.zeros(st["padded"], dtype=np.int16)   # pad -> row 0
            dstl = np.full(st["padded"], -999.0, dtype=np.float32)
            mh = h_c == h
            s_h, w_h, dl_h = s_c[mh], w_c[mh], dl_c[mh]
            order = np.argsort(w_h, kind="stable")
            s_h, w_h, dl_h = s_h[order], w_h[order], dl_h[order]
            # place each window's tokens at its global window start
            cnts = np.bincount(w_h, minlength=NWIN)
            pos = st["win_start"][w_h] + (np.arange(len(w_h))
                                          - np.repeat(np.cumsum(cnts) - cnts, cnts))
            tok[pos] = s_h.astype(np.int16)
            dstl[pos] = dl_h.astype(np.float32)
            core_tok[h] = (tok, dstl)
        # wrap idx tensors: [128, ncols]; within call k (2048 tokens ->
        # 128 cols), token j -> idxs[j % 16, k*128 + j // 16]
        idx_w = {}
        for h in (0, 1):
            tok = core_tok[h][0]
            st = streams[h]
            cols = []
            for k in range(st["ncalls"]):
                blk = tok[k * CALL:(k + 1) * CALL].reshape(128, 16).T  # [16,128]
                cols.append(blk)
            w16 = np.concatenate(cols, axis=1)          # [16, ncalls*128]
            idx_w[h] = np.tile(w16, (8, 1)).copy()      # [128, ncalls*128]
        # dstloc tensor [128, nseg]: segment s=(h, ch, w) -> column of
        # dloc - w*128 for the chunk's 128 tokens
        dstloc = np.full((128, nseg), -999.0, dtype=np.float32)
        for si, (h, ch, w) in enumerate(segs):
            dstl = core_tok[h][1]
            col = dstl[ch * 128:(ch + 1) * 128] - w * 128
            col[col < -500] = -999.0
            dstloc[:, si] = col
        per_core.append(dict(idx_lo=idx_w[0], idx_hi=idx_w[1], dstloc=dstloc))

    meta = dict(streams=streams, segs=segs, win_segs=win_segs, nseg=nseg)
    return meta, per_core


def _build_program(meta):
    import concourse.bass as bass
    import concourse.bacc as bacc
    import concourse.tile as tile
    import concourse.mybir as mybir

    f32 = mybir.dt.float32
    bf = mybir.dt.bfloat16
    i16 = mybir.dt.int16
    AF = mybir.ActivationFunctionType

    st_lo, st_hi = meta["streams"][0], meta["streams"][1]
    nseg = meta["nseg"]
    segs = meta["segs"]
    win_segs = meta["win_segs"]
    ncalls = {0: st_lo["ncalls"], 1: st_hi["ncalls"]}
    idx_cols = {h: ncalls[h] * 128 for h in (0, 1)}

    nc = bacc.Bacc("TRN2", target_bir_lowering=False, debug=False,
                   num_devices=NCORES)

    # ---- external I/O (per-core shards) ----
    xT = nc.dram_tensor("xT", [F_IN, NSH], bf, kind="ExternalInput")
    We = nc.dram_tensor("We", [F_IN, D], bf, kind="ExternalInput")
    beW = nc.dram_tensor("beW", [128, 2], f32, kind="ExternalInput")
    W1 = nc.dram_tensor("W1", [D, D], bf, kind="ExternalInput")
    W2 = nc.dram_tensor("W2", [D, D], bf, kind="ExternalInput")
    b1 = nc.dram_tensor("b1", [1, D], bf, kind="ExternalInput")
    b2 = nc.dram_tensor("b2", [1, D], bf, kind="ExternalInput")
    disw = nc.dram_tensor("disw", [128, NWIN], f32, kind="ExternalInput")
    sqd = nc.dram_tensor("sqd", [1, NSH], bf, kind="ExternalInput")
    iota = nc.dram_tensor("iota", [128, 128], f32, kind="ExternalInput")
    ident = nc.dram_tensor("ident", [128, 128], f32, kind="ExternalInput")
    identb = nc.dram_tensor("identb", [128, 128], bf, kind="ExternalInput")
    idx_lo = nc.dram_tensor("idx_lo", [128, idx_cols[0]], i16, kind="ExternalInput")
    idx_hi = nc.dram_tensor("idx_hi", [128, idx_cols[1]], i16, kind="ExternalInput")
    dstloc = nc.dram_tensor("dstloc", [128, nseg], f32, kind="ExternalInput")
    out = nc.dram_tensor("out", [NSH, D], f32, kind="ExternalOutput")

    with tile.TileContext(nc) as tc:
        with (
            tc.tile_pool(name="const", bufs=1) as constp,
            tc.tile_pool(name="hT", bufs=1) as hTp,
            tc.tile_pool(name="dram", bufs=1, space="DRAM") as dramp,
            tc.tile_pool(name="glo", bufs=3) as glop,
            tc.tile_pool(name="ghi", bufs=3) as ghip,
            tc.tile_pool(name="acc", bufs=4, space="PSUM") as accp,
            tc.tile_pool(name="tps", bufs=2, space="PSUM") as tpsp,
            tc.tile_pool(name="sg", bufs=3) as sgp,
            tc.tile_pool(name="ob", bufs=3) as obp,
            tc.tile_pool(name="sm", bufs=4) as smp,
        ):
            # ---- load constants ----
            xT_sb = constp.tile([F_IN, NSH], bf)
            We_sb = constp.tile([F_IN, D], bf)
            beW_sb = constp.tile([128, 2], f32)
            W1_sb = constp.tile([128, 2, D], bf)
            W2_sb = constp.tile([128, 2, D], bf)
            b1_sb = constp.tile([1, D], bf)
            b2_sb = constp.tile([1, D], bf)
            disw_sb = constp.tile([128, NWIN], f32)
            sqd_sb = constp.tile([1, NSH], bf)
            iota_sb = constp.tile([128, 128], f32)
            ident_sb = constp.tile([128, 128], f32)
            identb_sb = constp.tile([128, 128], bf)
            ilo_sb = constp.tile([128, idx_cols[0]], i16)
            ihi_sb = constp.tile([128, idx_cols[1]], i16)
            dstloc_sb = constp.tile([128, nseg], f32)

            nc.sync.dma_start(xT_sb[:], xT[:])
            nc.sync.dma_start(We_sb[:], We[:])
            nc.sync.dma_start(beW_sb[:], beW[:])
            nc.sync.dma_start(W1_sb[:, 0, :], W1[0:128, :])
            nc.sync.dma_start(W1_sb[:, 1, :], W1[128:256, :])
            nc.sync.dma_start(W2_sb[:, 0, :], W2[0:128, :])
            nc.sync.dma_start(W2_sb[:, 1, :], W2[128:256, :])
            nc.sync.dma_start(b1_sb[:], b1[:])
            nc.sync.dma_start(b2_sb[:], b2[:])
            nc.sync.dma_start(disw_sb[:], disw[:])
            nc.sync.dma_start(sqd_sb[:], sqd[:])
            nc.sync.dma_start(iota_sb[:], iota[:])
            nc.sync.dma_start(ident_sb[:], ident[:])
            nc.sync.dma_start(identb_sb[:], identb[:])
            nc.sync.dma_start(ilo_sb[:], idx_lo[:])
            nc.sync.dma_start(ihi_sb[:], idx_hi[:])
            nc.sync.dma_start(dstloc_sb[:], dstloc[:])

            h0T = hTp.tile([128, 2, NSH], bf)
            h1T = hTp.tile([128, 2, NSH], bf)

            # ---- embed: h0T = relu(We^T x^T + be), feature-major ----
            slabs = [(s, min(s + 512, NSH)) for s in range(0, NSH, 512)]
            for k in (0, 1):
                for (a, b) in slabs:
                    ps = accp.tile([128, 512], f32, tag="acc")
                    nc.tensor.matmul(ps[:, :b - a], lhsT=We_sb[:, k * 128:(k + 1) * 128],
                                     rhs=xT_sb[:, a:b], start=True, stop=True)
                    nc.scalar.activation(h0T[:, k, a:b], ps[:, :b - a], AF.Relu,
                                         bias=beW_sb[:, k:k + 1], scale=1.0)

            cc_in = {}
            cc_out = {}
            for l in (1, 2):
                cc_in[l] = {0: dramp.tile([AROWS, D], bf, name=f"ccinA{l}"),
                            1: dramp.tile([BROWS, D], bf, name=f"ccinB{l}")}
                cc_out[l] = {0: dramp.tile([NCORES * AROWS, D], bf,
                                           name=f"ccoutA{l}", addr_space="Shared"),
                             1: dramp.tile([NCORES * BROWS, D], bf,
                                           name=f"ccoutB{l}", addr_space="Shared")}
            accA = hTp.tile([128, NWIN, D], bf, name="accA")

            def dense(l, w, hT, W_sb):
                """g tile for window w of layer l -> SBUF + DMA to cc_in[l]."""
                ps = accp.tile([128, D], f32, tag="acc", name=f"dps{l}_{w}")
                for k in (0, 1):
                    nc.tensor.matmul(ps[:], lhsT=hT[:, k, w * 128:(w + 1) * 128],
                                     rhs=W_sb[:, k, :], start=(k == 0), stop=(k == 1))
                gt = obp.tile([128, D], bf, tag="ob", name=f"g{l}_{w}")
                nc.scalar.activation(gt[:], ps[:], AF.Copy, bias=0.0,
                                     scale=disw_sb[:, w:w + 1])
                if w < AWIN:
                    nc.sync.dma_start(cc_in[l][0][w * 128:(w + 1) * 128, :], gt[:])
                else:
                    ww = w - AWIN
                    nc.sync.dma_start(cc_in[l][1][ww * 128:(ww + 1) * 128, :], gt[:])

            def allgather(l, h):
                nc.gpsimd.collective_compute(
                    "AllGather", mybir.AluOpType.bypass,
                    replica_groups=[list(range(NCORES))],
                    ins=[cc_in[l][h][:]], outs=[cc_out[l][h][:]])

            for w in range(NWIN):
                dense(1, w, h0T, W1_sb)
                if w == AWIN - 1:
                    allgather(1, 0)
            allgather(1, 1)

            def edge_phase(l, b_sb, post_b):
                """Two-pass edge scatter for layer l.

                Pass A: psum = self + bias + A-half segments -> accA (bf16).
                Pass B: psum = B-half segments + I @ accA -> post_b(w, psum).
                Gather calls for half B sit after all half-A calls in the
                gpsimd queue, so AllGather of table half B hides under the
                half-A gathers."""
                gt_tiles = {0: {}, 1: {}}
                emitted = {0: 0, 1: 0}
                idx_sb = {0: ilo_sb, 1: ihi_sb}
                pool = {0: glop, 1: ghip}
                sizes = {0: st_lo["call_sizes"], 1: st_hi["call_sizes"]}

                def emit_call(h):
                    k = emitted[h]
                    nidx = sizes[h][k]
                    g = pool[h].tile([128, nidx // 128, D], bf, tag=f"g{h}",
                                     name=f"L{l}{'ab'[h]}{k}")
                    nc.gpsimd.dma_gather(
                        out_ap=g[:], in_ap=cc_out[l][h][:],
                        idxs_ap=idx_sb[h][:, k * 128:k * 128 + nidx // 16],
                        num_idxs=nidx, num_idxs_reg=nidx, elem_size=D,
                        single_packet=False)
                    gt_tiles[h][k] = g
                    emitted[h] += 1

                def seg_mms(h, w, ps, first_start, last_stop=False):
                    lst = win_segs[h][w]
                    for j, si in enumerate(lst):
                        _, ch, _ = segs[si]
                        call_k, cj = ch * 128 // CALL, (ch * 128 % CALL) // 128
                        S = smp.tile([128, 128], bf, tag="sm", name=f"S{l}_{si}")
                        nc.vector.tensor_tensor(
                            out=S[:],
                            in0=dstloc_sb[:, si:si + 1].to_broadcast([128, 128]),
                            in1=iota_sb[:],
                            op=mybir.AluOpType.is_equal)
                        nc.tensor.matmul(ps[:], lhsT=S[:],
                                         rhs=gt_tiles[h][call_k][:, cj, :],
                                         start=(first_start and j == 0),
                                         stop=(last_stop and j == len(lst) - 1))

                # ---- pass A ----
                for w in range(NWIN):
                    need = max((segs[si][1] * 128 // CALL + 1
                                for si in win_segs[0][w]), default=0)
                    while emitted[0] < need:
                        emit_call(0)
                    ps = accp.tile([128, D], f32, tag="acc", name=f"pa{l}_{w}")
                    sgt = sgp.tile([128, D], bf, tag="sg", name=f"sg{l}_{w}")
                    if w < AWIN:
                        nc.sync.dma_start(sgt[:], cc_in[l][0][w * 128:(w + 1) * 128, :])
                    else:
                        ww = w - AWIN
                        nc.sync.dma_start(sgt[:], cc_in[l][1][ww * 128:(ww + 1) * 128, :])
                    nc.tensor.matmul(ps[:], lhsT=identb_sb[:], rhs=sgt[:],
                                     start=True, stop=False)
                    nc.tensor.matmul(ps[:], lhsT=sqd_sb[0:1, w * 128:(w + 1) * 128],
                                     rhs=b_sb[0:1, :], start=False,
                                     stop=len(win_segs[0][w]) == 0)
                    seg_mms(0, w, ps, False, last_stop=True)
                    nc.scalar.copy(accA[:, w, :], ps[:])
                # ---- pass B ----
                for w in range(NWIN):
                    need = max((segs[si][1] * 128 // CALL + 1
                                for si in win_segs[1][w]), default=0)
                    while emitted[1] < need:
                        emit_call(1)
                    ps = accp.tile([128, D], f32, tag="acc", name=f"pb{l}_{w}")
                    seg_mms(1, w, ps, True, last_stop=False)
                    nc.tensor.matmul(ps[:], lhsT=identb_sb[:], rhs=accA[:, w, :],
                                     start=len(win_segs[1][w]) == 0, stop=True)
                    post_b(w, ps)

            # ---- layer 1: edge phase -> h1T (transposed) + dense2 ----
            def l1_post(w, ps):
                ot = obp.tile([128, D], f32, tag="ob", name=f"h1_{w}")
                nc.scalar.activation(ot[:], ps[:], AF.Relu, bias=0.0,
                                     scale=disw_sb[:, w:w + 1])
                for k in (0, 1):
                    tp = tpsp.tile([128, 128], f32, tag="tp", name=f"tp{w}_{k}")
                    nc.tensor.transpose(tp[:], ot[:, k * 128:(k + 1) * 128],
                                        ident_sb[:])
                    nc.scalar.copy(h1T[:, k, w * 128:(w + 1) * 128], tp[:])
                dense(2, w, h1T, W2_sb)
                if w == AWIN - 1:
                    allgather(2, 0)
                elif w == NWIN - 1:
                    allgather(2, 1)

            edge_phase(1, b1_sb, l1_post)

            # ---- layer 2 ----
            def l2_post(w, ps):
                ot = obp.tile([128, D], f32, tag="ob", name=f"o_{w}")
                nc.scalar.activation(ot[:], ps[:], AF.Copy, bias=0.0,
                                     scale=disw_sb[:, w:w + 1])
                nc.sync.dma_start(out[w * 128:(w + 1) * 128, :], ot[:])

            edge_phase(2, b2_sb, l2_post)

    nc.compile()
    return nc


def _prep_inputs(x, edge_index, W_embed, b_embed, W1, b1, W2, b2):
    src0, dst0 = np.asarray(edge_index[0]).astype(np.int64), \
        np.asarray(edge_index[1]).astype(np.int64)

    # degree-balanced node relabeling: deal degree-sorted nodes round-robin
    # into the 392 global windows so per-window edge counts (and the
    # max-over-cores stream padding) are nearly uniform.
    deg_orig = np.zeros(NTAB, dtype=np.int64)
    np.add.at(deg_orig, dst0, 1)
    order = np.argsort(-deg_orig, kind="stable")
    nwin_g = NTAB // 128
    ar = np.arange(NTAB)
    newid = np.empty(NTAB, dtype=np.int64)
    newid[order] = (ar % nwin_g) * 128 + ar // nwin_g
    src, dst = newid[src0], newid[dst0]
    meta, per_core = _edge_plan(src, dst)

    deg_d = 1.0 + np.zeros(NTAB, dtype=np.float64)
    np.add.at(deg_d, dst, 1)
    deg = deg_d
    dis = (1.0 / np.sqrt(deg)).astype(np.float32)
    sq = np.sqrt(deg).astype(np.float32)

    xpad = np.zeros((NTAB, F_IN), dtype=np.float32)
    xpad[newid[:N_NODES]] = x
    xT_full = np.ascontiguousarray(xpad.T)            # [11, NTAB]

    beW = np.asarray(b_embed, dtype=np.float32).reshape(2, 128).T.copy()  # [128,2]
    iota = np.tile(np.arange(128, dtype=np.float32), (128, 1))
    ident = np.eye(128, dtype=np.float32)

    in_maps = []
    for c in range(NCORES):
        sl = slice(c * NSH, (c + 1) * NSH)
        disw = dis[sl].reshape(NWIN, 128).T.copy()    # [128, NWIN]
        in_maps.append({
            "xT": np.ascontiguousarray(xT_full[:, sl]).astype(BF16),
            "We": np.asarray(W_embed, dtype=np.float32).astype(BF16),
            "beW": beW,
            "W1": np.asarray(W1, dtype=np.float32).astype(BF16),
            "W2": np.asarray(W2, dtype=np.float32).astype(BF16),
            "b1": np.asarray(b1, dtype=np.float32).reshape(1, D).astype(BF16),
            "b2": np.asarray(b2, dtype=np.float32).reshape(1, D).astype(BF16),
            "disw": disw,
            "sqd": sq[sl].reshape(1, NSH).astype(BF16),
            "iota": iota,
            "ident": ident,
            "identb": ident.astype(BF16),
            "idx_lo": per_core[c]["idx_lo"],
            "idx_hi": per_core[c]["idx_hi"],
            "dstloc": per_core[c]["dstloc"],
        })
    return meta, in_maps, newid


def kernel(x, edge_index, W_embed, b_embed, W1, b1, W2, b2, _trace=False):
    from concourse.bass_utils import run_bass_kernel_spmd

    meta, in_maps, newid = _prep_inputs(x, edge_index, W_embed, b_embed,
                                        W1, b1, W2, b2)
    key = (meta["streams"][0]["ncalls"], meta["streams"][1]["ncalls"],
           meta["nseg"], tuple(meta["streams"][0]["lens"].tolist()),
           tuple(meta["streams"][1]["lens"].tolist()))
    if key not in _CACHE:
        _CACHE.clear()
        _CACHE[key] = _build_program(meta)
    nc = _CACHE[key]

    res = run_bass_kernel_spmd(nc, in_maps, core_ids=list(range(NCORES)),
                               trace=_trace)
    full = np.concatenate([res.results[c]["out"] for c in range(NCORES)], axis=0)
    kernel._last_exec_ns = res.exec_time_ns
    return full[newid[:N_NODES]].astype(np.float32)


# revision 13
# speedup vs baseline: 1.1502x; 1.0788x over previous
"""Distributed Trainium2 (Bass) kernel for nn_AtomEmbedder (2-layer GCN + embed).

Strategy (8 NeuronCores, node-sharded):
  - Nodes padded to 50176 = 8 * 6272; core c owns dst rows [c*6272, (c+1)*6272).
  - h0 = relu(x @ We + be) computed feature-major per shard (no transposes).
  - Per GCN layer l:
      g = dis * (h @ Wl)  (node-major shard, dense matmuls on PE)
      AllGather g -> full 50176-row table in DRAM
      edge scatter: for each 128-dst-node window, accumulate in PSUM
        sum_e g[src_e] via dma_gather (token stream sorted by window) +
        one-hot matmul (S^T @ gathered), plus self-loop (identity matmul on
        own shard rows) and bias folded as a K=1 matmul of sqrt(deg) x b.
      epilogue: out = act(dis * psum) in one ScalarE op per window.
  - Layer-1 windows are transposed on PE into feature-major h1T so layer-2
    dense matmuls start immediately (hidden under layer-1 gathers).

The token gather is Q7-descriptor-rate-bound (~8.5 ns/token); everything
else (PE matmuls, DVE one-hot builds, ACT epilogues, dense DMA, AllGather)
is scheduled to hide underneath it.
"""

import numpy as np
import ml_dtypes

BF16 = ml_dtypes.bfloat16
N_NODES = 50000
N_EDGES = 300000
F_IN = 11
D = 256
NCORES = 8
NSH = 6272            # nodes per core (50176 total, padded)
NTAB = NCORES * NSH   # 50176
HALF = NTAB // 2
AWIN = 25             # windows per core in table half A
AROWS = AWIN * 128    # 3200 rows/core  -> A table 25600 rows (< 32768, int16-safe)
BROWS = NSH - AROWS   # 3072 rows/core  -> B table 24576 rows
NWIN = NSH // 128     # 49 windows per core
CALL = 2048           # tokens per dma_gather call
P = 128

_CACHE = {}


def _edge_plan(src, dst):
    """Build the SPMD-uniform token-stream / segment structure plus per-core
    index data. Returns (meta, per_core_arrays)."""
    src = src.astype(np.int64)
    dst = dst.astype(np.int64)
    core = dst // NSH
    dloc = dst % NSH
    win = dloc // 128
    rsrc = src % NSH
    csrc = src // NSH
    half = (rsrc >= AROWS).astype(np.int64)
    tokval = np.where(half == 0, csrc * AROWS + rsrc,
                      csrc * BROWS + (rsrc - AROWS))

    # counts[c, w, h]
    counts = np.zeros((NCORES, NWIN, 2), dtype=np.int64)
    np.add.at(counts, (core, win, half), 1)
    gcnt = counts.max(axis=0)              # [NWIN, 2] uniform per-window counts

    # stream layout per half: windows in order, gcnt tokens each
    streams = {}
    for h in (0, 1):
        lens = gcnt[:, h]
        total = int(lens.sum())
        ncalls = max(1, -(-total // CALL))
        padded = ncalls * CALL
        last = total - (ncalls - 1) * CALL
        call_sizes = [CALL] * (ncalls - 1) + [max(128, -(-last // 128) * 128)]
        win_start = np.zeros(NWIN + 1, dtype=np.int64)
        win_start[1:] = np.cumsum(lens)
        streams[h] = dict(lens=lens, total=total, ncalls=ncalls, padded=padded,
                          win_start=win_start, call_sizes=call_sizes)

    # segments: (half, chunk, window) for every chunk x window overlap
    segs = []          # list of (half, chunk_idx, window)
    win_segs = {0: [[] for _ in range(NWIN)], 1: [[] for _ in range(NWIN)]}
    for h in (0, 1):
        ws = streams[h]["win_start"]
        nchunk = streams[h]["padded"] // 128
        for w in range(NWIN):
            a, b = int(ws[w]), int(ws[w + 1])
            if a == b:
                continue
            for ch in range(a // 128, (b - 1) // 128 + 1):
                win_segs[h][w].append(len(segs))
                segs.append((h, ch, w))
        streams[h]["nchunk"] = nchunk
    nseg = len(segs)

    # per-core data: token indices (int16, per half) and dstloc columns
    per_core = []
    for c in range(NCORES):
        m = core == c
        s_c, w_c, h_c, dl_c = tokval[m], win[m], half[m], dloc[m]
        core_tok = {}
        for h in (0, 1):
            st = streams[h]
            tok = np.zeros(st["padded"], dtype=np.int16)   # pad -> row 0
            dstl = np.full(st["padded"], -999.0, dtype=np.float32)
            mh = h_c == h
            s_h, w_h, dl_h = s_c[mh], w_c[mh], dl_c[mh]
            order = np.argsort(w_h, kind="stable")
            s_h, w_h, dl_h = s_h[order], w_h[order], dl_h[order]
            # place each window's tokens at its global window start
            cnts = np.bincount(w_h, minlength=NWIN)
            pos = st["win_start"][w_h] + (np.arange(len(w_h))
                                          - np.repeat(np.cumsum(cnts) - cnts, cnts))
            tok[pos] = s_h.astype(np.int16)
            dstl[pos] = dl_h.astype(np.float32)
            core_tok[h] = (tok, dstl)
        # wrap idx tensors: [128, ncols]; within call k (2048 tokens ->
        # 128 cols), token j -> idxs[j % 16, k*128 + j // 16]
        idx_w = {}
        for h in (0, 1):
            tok = core_tok[h][0]
            st = streams[h]
            cols = []
            for k in range(st["ncalls"]):
                blk = tok[k * CALL:(k + 1) * CALL].reshape(128, 16).T  # [16,128]
                cols.append(blk)
            w16 = np.concatenate(cols, axis=1)          # [16, ncalls*128]
            idx_w[h] = np.tile(w16, (8, 1)).copy()      # [128, ncalls*128]
        # dstloc tensor [128, nseg]: segment s=(h, ch, w) -> column of
        # dloc - w*128 for the chunk's 128 tokens
        dstloc = np.full((128, nseg), -999.0, dtype=np.float32)
        for si, (h, ch, w) in enumerate(segs):
            dstl = core_tok[h][1]
            col = dstl[ch * 128:(ch + 1) * 128] - w * 128
            col[col < -500] = -999.0
            dstloc[:, si] = col
        per_core.append(dict(idx_lo=idx_w[0], idx_hi=idx_w[1], dstloc=dstloc))

    meta = dict(streams=streams, segs=segs, win_segs=win_segs, nseg=nseg)
    return meta, per_core


def _build_program(meta):
    import concourse.bass as bass
    import concourse.bacc as bacc
    import concourse.tile as tile
    import concourse.mybir as mybir

    f32 = mybir.dt.float32
    bf = mybir.dt.bfloat16
    i16 = mybir.dt.int16
    AF = mybir.ActivationFunctionType

    st_lo, st_hi = meta["streams"][0], meta["streams"][1]
    nseg = meta["nseg"]
    segs = meta["segs"]
    win_segs = meta["win_segs"]
    ncalls = {0: st_lo["ncalls"], 1: st_hi["ncalls"]}
    idx_cols = {h: ncalls[h] * 128 for h in (0, 1)}

    nc = bacc.Bacc("TRN2", target_bir_lowering=False, debug=False,
                   num_devices=NCORES)

    # ---- external I/O (per-core shards) ----
    xT = nc.dram_tensor("xT", [F_IN, NSH], bf, kind="ExternalInput")
    We = nc.dram_tensor("We", [F_IN, D], bf, kind="ExternalInput")
    beW = nc.dram_tensor("beW", [128, 2], f32, kind="ExternalInput")
    W1 = nc.dram_tensor("W1", [D, D], bf, kind="ExternalInput")
    W2 = nc.dram_tensor("W2", [D, D], bf, kind="ExternalInput")
    b1 = nc.dram_tensor("b1", [1, D], bf, kind="ExternalInput")
    b2 = nc.dram_tensor("b2", [1, D], bf, kind="ExternalInput")
    disw = nc.dram_tensor("disw", [128, NWIN], f32, kind="ExternalInput")
    sqd = nc.dram_tensor("sqd", [1, NSH], bf, kind="ExternalInput")
    iota = nc.dram_tensor("iota", [128, 128], f32, kind="ExternalInput")
    ident = nc.dram_tensor("ident", [128, 128], f32, kind="ExternalInput")
    identb = nc.dram_tensor("identb", [128, 128], bf, kind="ExternalInput")
    idx_lo = nc.dram_tensor("idx_lo", [128, idx_cols[0]], i16, kind="ExternalInput")
    idx_hi = nc.dram_tensor("idx_hi", [128, idx_cols[1]], i16, kind="ExternalInput")
    dstloc = nc.dram_tensor("dstloc", [128, nseg], f32, kind="ExternalInput")
    out = nc.dram_tensor("out", [NSH, D], f32, kind="ExternalOutput")

    with tile.TileContext(nc) as tc:
        with (
            tc.tile_pool(name="const", bufs=1) as constp,
            tc.tile_pool(name="hT", bufs=1) as hTp,
            tc.tile_pool(name="dram", bufs=1, space="DRAM") as dramp,
            tc.tile_pool(name="glo", bufs=3) as glop,
            tc.tile_pool(name="ghi", bufs=3) as ghip,
            tc.tile_pool(name="acc", bufs=4, space="PSUM") as accp,
            tc.tile_pool(name="tps", bufs=2, space="PSUM") as tpsp,
            tc.tile_pool(name="sg", bufs=3) as sgp,
            tc.tile_pool(name="ob", bufs=3) as obp,
            tc.tile_pool(name="sm", bufs=4) as smp,
        ):
            # ---- load constants ----
            xT_sb = constp.tile([F_IN, NSH], bf)
            We_sb = constp.tile([F_IN, D], bf)
            beW_sb = constp.tile([128, 2], f32)
            W1_sb = constp.tile([128, 2, D], bf)
            W2_sb = constp.tile([128, 2, D], bf)
            b1_sb = constp.tile([1, D], bf)
            b2_sb = constp.tile([1, D], bf)
            disw_sb = constp.tile([128, NWIN], f32)
            sqd_sb = constp.tile([1, NSH], bf)
            iota_sb = constp.tile([128, 128], f32)
            ident_sb = constp.tile([128, 128], f32)
            identb_sb = constp.tile([128, 128], bf)
            ilo_sb = constp.tile([128, idx_cols[0]], i16)
            ihi_sb = constp.tile([128, idx_cols[1]], i16)
            dstloc_sb = constp.tile([128, nseg], f32)

            nc.sync.dma_start(xT_sb[:], xT[:])
            nc.sync.dma_start(We_sb[:], We[:])
            nc.sync.dma_start(beW_sb[:], beW[:])
            nc.sync.dma_start(W1_sb[:, 0, :], W1[0:128, :])
            nc.sync.dma_start(W1_sb[:, 1, :], W1[128:256, :])
            nc.sync.dma_start(W2_sb[:, 0, :], W2[0:128, :])
            nc.sync.dma_start(W2_sb[:, 1, :], W2[128:256, :])
            nc.sync.dma_start(b1_sb[:], b1[:])
            nc.sync.dma_start(b2_sb[:], b2[:])
            nc.sync.dma_start(disw_sb[:], disw[:])
            nc.sync.dma_start(sqd_sb[:], sqd[:])
            nc.sync.dma_start(iota_sb[:], iota[:])
            nc.sync.dma_start(ident_sb[:], ident[:])
            nc.sync.dma_start(identb_sb[:], identb[:])
            nc.sync.dma_start(ilo_sb[:], idx_lo[:])
            nc.sync.dma_start(ihi_sb[:], idx_hi[:])
            nc.sync.dma_start(dstloc_sb[:], dstloc[:])

            h0T = hTp.tile([128, 2, NSH], bf)
            h1T = hTp.tile([128, 2, NSH], bf)

            # ---- embed: h0T = relu(We^T x^T + be), feature-major ----
            slabs = [(s, min(s + 512, NSH)) for s in range(0, NSH, 512)]
            for k in (0, 1):
                for (a, b) in slabs:
                    ps = accp.tile([128, 512], f32, tag="acc")
                    nc.tensor.matmul(ps[:, :b - a], lhsT=We_sb[:, k * 128:(k + 1) * 128],
                                     rhs=xT_sb[:, a:b], start=True, stop=True)
                    nc.scalar.activation(h0T[:, k, a:b], ps[:, :b - a], AF.Relu,
                                         bias=beW_sb[:, k:k + 1], scale=1.0)

            cc_in = {}
            cc_out = {}
            for l in (1, 2):
                cc_in[l] = {0: dramp.tile([AROWS, D], bf, name=f"ccinA{l}"),
                            1: dramp.tile([BROWS, D], bf, name=f"ccinB{l}")}
                cc_out[l] = {0: dramp.tile([NCORES * AROWS, D], bf,
                                           name=f"ccoutA{l}", addr_space="Shared"),
                             1: dramp.tile([NCORES * BROWS, D], bf,
                                           name=f"ccoutB{l}", addr_space="Shared")}
            accA = hTp.tile([128, NWIN, D], bf, name="accA")

            def dense(l, w, hT, W_sb):
                """g tile for window w of layer l -> SBUF + DMA to cc_in[l]."""
                ps = accp.tile([128, D], f32, tag="acc", name=f"dps{l}_{w}")
                for k in (0, 1):
                    nc.tensor.matmul(ps[:], lhsT=hT[:, k, w * 128:(w + 1) * 128],
                                     rhs=W_sb[:, k, :], start=(k == 0), stop=(k == 1))
                gt = obp.tile([128, D], bf, tag="ob", name=f"g{l}_{w}")
                nc.scalar.activation(gt[:], ps[:], AF.Copy, bias=0.0,
                                     scale=disw_sb[:, w:w + 1])
                if w < AWIN:
                    nc.sync.dma_start(cc_in[l][0][w * 128:(w + 1) * 128, :], gt[:])
                else:
                    ww = w - AWIN
                    nc.sync.dma_start(cc_in[l][1][ww * 128:(ww + 1) * 128, :], gt[:])

            def allgather(l, h):
                nc.gpsimd.collective_compute(
                    "AllGather", mybir.AluOpType.bypass,
                    replica_groups=[list(range(NCORES))],
                    ins=[cc_in[l][h][:]], outs=[cc_out[l][h][:]])

            for w in range(NWIN):
                dense(1, w, h0T, W1_sb)
                if w == AWIN - 1:
                    allgather(1, 0)
            allgather(1, 1)

            def edge_phase(l, b_sb, post_b):
                """Two-pass edge scatter for layer l.

                Pass A: psum = self + bias + A-half segments -> accA (bf16).
                Pass B: psum = B-half segments + I @ accA -> post_b(w, psum).
                Gather calls for half B sit after all half-A calls in the
                gpsimd queue, so AllGather of table half B hides under the
                half-A gathers."""
                gt_tiles = {0: {}, 1: {}}
                emitted = {0: 0, 1: 0}
                idx_sb = {0: ilo_sb, 1: ihi_sb}
                pool = {0: glop, 1: ghip}
                sizes = {0: st_lo["call_sizes"], 1: st_hi["call_sizes"]}

                def emit_call(h):
                    k = emitted[h]
                    nidx = sizes[h][k]
                    g = pool[h].tile([128, nidx // 128, D], bf, tag=f"g{h}",
                                     name=f"L{l}{'ab'[h]}{k}")
                    nc.gpsimd.dma_gather(
                        out_ap=g[:], in_ap=cc_out[l][h][:],
                        idxs_ap=idx_sb[h][:, k * 128:k * 128 + nidx // 16],
                        num_idxs=nidx, num_idxs_reg=nidx, elem_size=D,
                        single_packet=False)
                    gt_tiles[h][k] = g
                    emitted[h] += 1

                def seg_mms(h, w, ps, first_start, last_stop=False):
                    lst = win_segs[h][w]
                    for j, si in enumerate(lst):
                        _, ch, _ = segs[si]
                        call_k, cj = ch * 128 // CALL, (ch * 128 % CALL) // 128
                        S = smp.tile([128, 128], bf, tag="sm", name=f"S{l}_{si}")
                        nc.vector.tensor_tensor(
                            out=S[:],
                            in0=dstloc_sb[:, si:si + 1].to_broadcast([128, 128]),
                            in1=iota_sb[:],
                            op=mybir.AluOpType.is_equal)
                        nc.tensor.matmul(ps[:], lhsT=S[:],
                                         rhs=gt_tiles[h][call_k][:, cj, :],
                                         start=(first_start and j == 0),
                                         stop=(last_stop and j == len(lst) - 1))

                # ---- pass A ----
                for w in range(NWIN):
                    need = max((segs[si][1] * 128 // CALL + 1
                                for si in win_segs[0][w]), default=0)
                    while emitted[0] < need:
                        emit_call(0)
                    ps = accp.tile([128, D], f32, tag="acc", name=f"pa{l}_{w}")
                    sgt = sgp.tile([128, D], bf, tag="sg", name=f"sg{l}_{w}")
                    if w < AWIN:
                        nc.sync.dma_start(sgt[:], cc_in[l][0][w * 128:(w + 1) * 128, :])
                    else:
                        ww = w - AWIN
                        nc.sync.dma_start(sgt[:], cc_in[l][1][ww * 128:(ww + 1) * 128, :])
                    nc.tensor.matmul(ps[:], lhsT=identb_sb[:], rhs=sgt[:],
                                     start=True, stop=False)
                    nc.tensor.matmul(ps[:], lhsT=sqd_sb[0:1, w * 128:(w + 1) * 128],
                                     rhs=b_sb[0:1, :], start=False,
                                     stop=len(win_segs[0][w]) == 0)
                    seg_mms(0, w, ps, False, last_stop=True)
                    nc.scalar.copy(accA[:, w, :], ps[:])
                # ---- pass B ----
                for w in range(NWIN):
                    need = max((segs[si][1] * 128 // CALL + 1
                                for si in win_segs[1][w]), default=0)
                    while emitted[1] < need:
                        emit_call(1)
                    ps = accp.tile([128, D], f32, tag="acc", name=f"pb{l}_{w}")
                    seg_mms(1, w, ps, True, last_stop=False)
                    nc.tensor.matmul(ps[:], lhsT=identb_sb[:], rhs=accA[:, w, :],
                                     start=len(win_segs[1][w]) == 0, stop=True)
                    post_b(w, ps)

            # ---- layer 1: edge phase -> h1T (transposed) + dense2 ----
            def l1_post(w, ps):
                ot = obp.tile([128, D], f32, tag="ob", name=f"h1_{w}")
                nc.scalar.activation(ot[:], ps[:], AF.Relu, bias=0.0,
                                     scale=disw_sb[:, w:w + 1])
                for k in (0, 1):
                    tp = tpsp.tile([128, 128], f32, tag="tp", name=f"tp{w}_{k}")
                    nc.tensor.transpose(tp[:], ot[:, k * 128:(k + 1) * 128],
                                        ident_sb[:])
                    nc.scalar.copy(h1T[:, k, w * 128:(w + 1) * 128], tp[:])
                dense(2, w, h1T, W2_sb)
                if w == AWIN - 1:
                    allgather(2, 0)
                elif w == NWIN - 1:
                    allgather(2, 1)

            edge_phase(1, b1_sb, l1_post)

            # ---- layer 2 ----
            def l2_post(w, ps):
                ot = obp.tile([128, D], f32, tag="ob", name=f"o_{w}")
                nc.scalar.activation(ot[:], ps[:], AF.Copy, bias=0.0,
                                     scale=disw_sb[:, w:w + 1])
                nc.sync.dma_start(out[w * 128:(w + 1) * 128, :], ot[:])

            edge_phase(2, b2_sb, l2_post)

    nc.compile()
    return nc


def _prep_inputs(x, edge_index, W_embed, b_embed, W1, b1, W2, b2):
    src0, dst0 = np.asarray(edge_index[0]).astype(np.int64), \
        np.asarray(edge_index[1]).astype(np.int64)

    newid = np.arange(NTAB, dtype=np.int64)
    src, dst = src0, dst0
    meta, per_core = _edge_plan(src, dst)

    deg_d = 1.0 + np.zeros(NTAB, dtype=np.float64)
    np.add.at(deg_d, dst, 1)
    deg = deg_d
    dis = (1.0 / np.sqrt(deg)).astype(np.float32)
    sq = np.sqrt(deg).astype(np.float32)

    xpad = np.zeros((NTAB, F_IN), dtype=np.float32)
    xpad[newid[:N_NODES]] = x
    xT_full = np.ascontiguousarray(xpad.T)            # [11, NTAB]

    beW = np.asarray(b_embed, dtype=np.float32).reshape(2, 128).T.copy()  # [128,2]
    iota = np.tile(np.arange(128, dtype=np.float32), (128, 1))
    ident = np.eye(128, dtype=np.float32)

    in_maps = []
    for c in range(NCORES):
        sl = slice(c * NSH, (c + 1) * NSH)
        disw = dis[sl].reshape(NWIN, 128).T.copy()    # [128, NWIN]
        in_maps.append({
            "xT": np.ascontiguousarray(xT_full[:, sl]).astype(BF16),
            "We": np.asarray(W_embed, dtype=np.float32).astype(BF16),
            "beW": beW,
            "W1": np.asarray(W1, dtype=np.float32).astype(BF16),
            "W2": np.asarray(W2, dtype=np.float32).astype(BF16),
            "b1": np.asarray(b1, dtype=np.float32).reshape(1, D).astype(BF16),
            "b2": np.asarray(b2, dtype=np.float32).reshape(1, D).astype(BF16),
            "disw": disw,
            "sqd": sq[sl].reshape(1, NSH).astype(BF16),
            "iota": iota,
            "ident": ident,
            "identb": ident.astype(BF16),
            "idx_lo": per_core[c]["idx_lo"],
            "idx_hi": per_core[c]["idx_hi"],
            "dstloc": per_core[c]["dstloc"],
        })
    return meta, in_maps, newid


def kernel(x, edge_index, W_embed, b_embed, W1, b1, W2, b2, _trace=False):
    from concourse.bass_utils import run_bass_kernel_spmd

    meta, in_maps, newid = _prep_inputs(x, edge_index, W_embed, b_embed,
                                        W1, b1, W2, b2)
    key = (meta["streams"][0]["ncalls"], meta["streams"][1]["ncalls"],
           meta["nseg"], tuple(meta["streams"][0]["lens"].tolist()),
           tuple(meta["streams"][1]["lens"].tolist()))
    if key not in _CACHE:
        _CACHE.clear()
        _CACHE[key] = _build_program(meta)
    nc = _CACHE[key]

    res = run_bass_kernel_spmd(nc, in_maps, core_ids=list(range(NCORES)),
                               trace=_trace)
    full = np.concatenate([res.results[c]["out"] for c in range(NCORES)], axis=0)
    kernel._last_exec_ns = res.exec_time_ns
    return full[newid[:N_NODES]].astype(np.float32)
